# revision 54
# baseline (speedup 1.0000x reference)
"""CAM_Module (channel attention) Trainium2 Bass kernel (all-bf16 variant).

x is SWDGE cast-loaded straight to bf16 (the only resident copy): every PE
path (transpose 1 cyc/row, MM1, MM2) runs full-rate bf16, and the residual
carries a one-time bf16 rounding (~2.3e-3 rel vs the 2e-2 gate).

x: (16, 512, 64, 64) f32, gamma: (1,) f32
  xf = x.reshape(B, C, N)           N = 4096
  energy = xf @ xf^T                (B, C, C)
  att = softmax(max(energy) - energy, axis=-1)   == softmax(-energy) (shift-invariant)
  out = gamma * (att @ xf) + x

Sharding: data-parallel over batch, 2 batches per core on 8 cores.

Per-core pipeline (per batch):
  - SWDGE cast-load x -> SBUF bf16 (chunked per c-tile so compute starts
    before the full batch arrives)
  - PE transpose-mode (bf16, 1 cyc/row): xf^T chunks -> PSUM -> evac (ACT
    2/3 / DVE 1/3) -> SBUF bf16
  - MM1 (bf16): energy j>=i blocks accumulated over 32 k-chunks into 4
    PSUM banks; lower-triangle blocks mirrored via PE transpose
  - softmax: DVE row-min, ACT exp(min - e) with row-sum accumulation,
    DVE reciprocal, scale by gamma/Z
  - PE transpose att -> att^T (bf16)
  - MM2 (bf16): att^T.T @ xf accumulated over 4 j-chunks, DVE adds the
    (bf16-rounded) residual x
  - HWDGE store f32

Measured on HW (257-rep loop slope, 8 cores): 153.2 / 160.9 us across two
runs of this exact program (~5% run-to-run HW variance), vs 187.4 us for
the session-start baseline measured the same way. Rejected variants, all
HW-measured: fp8 DoubleRow matmuls (~1.7-1.9x slower than the cost model
predicts: un-hidden 256-col LDWEIGHTS per DR matmul, and fp8
transpose-mode's mandatory step-2 strided PSUM writes -> 185-202 us);
fine-grained or coarse-block emission interleave of one batch's MM2 with
the other's transposes (161-174 us); merged 3D HWDGE loads (167 us);
folding gamma into the MM2 evac + deeper prefix (160 us); paired
transpose evacs (164 us). The remaining gap to the ~94 us DMA floor is
the serial head (batch-0 load) and tail (batch-1 MM2+stores).
"""

import sys

if "/opt/trn_rl_repo" not in sys.path:
    sys.path.insert(0, "/opt/trn_rl_repo")

from contextlib import ExitStack

import numpy as np

import concourse.bass as bass
import concourse.tile as tile
from concourse import bacc, mybir
from concourse.bass_utils import run_bass_kernel_spmd
from concourse.masks import make_identity

N_CORES = 8
B, C, H, W = 16, 512, 64, 64
N = H * W                    # 4096
BPC = B // N_CORES           # batches per core = 2
CT = C // 128                # 4 c-tiles
KT = N // 128                # 32 k-chunks (transposed layout)
NCH = N // 512               # 8 moving chunks for MM2

F32 = mybir.dt.float32
F32R = mybir.dt.float32r
BF16 = mybir.dt.bfloat16


def _build_nc(reps=1, upto="full"):
    nc = bacc.Bacc("TRN2", target_bir_lowering=False, debug=False,
                   num_devices=N_CORES)
    x_d = nc.dram_tensor("x", [BPC, C, N], F32, kind="ExternalInput").ap()
    g_d = nc.dram_tensor("gamma", [1], F32, kind="ExternalInput").ap()
    o_d = nc.dram_tensor("out", [BPC, C, N], F32, kind="ExternalOutput").ap()

    with tile.TileContext(nc) as tc, ExitStack() as ctx:
        xf_pool = ctx.enter_context(tc.tile_pool(name="xf", bufs=BPC))
        xfT_pool = ctx.enter_context(tc.tile_pool(name="xfT", bufs=12))
        s_pool = ctx.enter_context(tc.tile_pool(name="s", bufs=CT))
        att_pool = ctx.enter_context(tc.tile_pool(name="att", bufs=CT))
        attT_pool = ctx.enter_context(tc.tile_pool(name="attT", bufs=CT))
        out_pool = ctx.enter_context(tc.tile_pool(name="outp", bufs=6))
        stat_pool = ctx.enter_context(tc.tile_pool(name="stat", bufs=4 * CT))
        one_pool = ctx.enter_context(tc.tile_pool(name="one", bufs=1))
        pT = ctx.enter_context(tc.tile_pool(name="pT", bufs=2, space="PSUM"))
        pE = ctx.enter_context(tc.tile_pool(name="pE", bufs=CT, space="PSUM"))
        pO = ctx.enter_context(tc.tile_pool(name="pO", bufs=2, space="PSUM"))

        # identity for PE transpose-mode (f32r so dtypes match the data)
        ident_f = one_pool.tile([128, 128], F32, tag="idf")
        make_identity(nc, ident_f[:])
        ident = one_pool.tile([128, 128], BF16, tag="idb")
        nc.vector.tensor_copy(ident[:], ident_f[:])

        # broadcast gamma to all 128 partitions via K=1 matmul with ones
        g_sb = one_pool.tile([1, 1], F32, tag="gsb")
        nc.sync.dma_start(g_sb[:], g_d.rearrange("(a b) -> a b", a=1))
        ones = one_pool.tile([1, 128], F32, tag="ones")
        nc.vector.memset(ones[:], 1.0)
        pG = pT.tile([128, 1], F32, tag="pt", name="pG")
        nc.tensor.matmul(pG[:], ones[:], g_sb[:], start=True, stop=True)
        g_bc = one_pool.tile([128, 1], F32, tag="gbc")
        nc.vector.tensor_copy(g_bc[:], pG[:])

        loop_ctx = tc.For_i(0, reps, 1) if reps > 1 else None
        if loop_ctx is not None:
            ctx.enter_context(loop_ctx)

        # per-c-tile load chunks: a small first chunk so the pipeline
        # starts early, bigger ones later (amortize SWDGE fixed cost)
        CHUNKS = [(0, 512), (512, 512), (1024, 1024), (2048, 1024),
                  (3072, 1024)]

        def chunk_of(col):
            for i, (off, w) in enumerate(CHUNKS):
                if off <= col < off + w:
                    return i, col - off
            raise AssertionError(col)

        st = [dict() for _ in range(BPC)]

        def emit_loads(b):
            # one bf16 tile per batch; each chunk is a single 3D SWDGE
            # cast-DMA covering all 4 c-tiles: Q7 descgen cost is per-DMA
            # dominated, so 5 DMAs stream at HBM pace where 20 were
            # descgen-paced
            s = st[b]
            t = xf_pool.tile([128, CT, N], BF16, tag="xf", name=f"xf_{b}")
            src = x_d[b].rearrange("(ct p) n -> p ct n", p=128)
            for q in range(len(CHUNKS)):
                off, w = CHUNKS[q]
                nc.gpsimd.dma_start(t[:, :, off:off + w],
                                    src[:, :, off:off + w])
            s["xf"] = t

        def xf_slice(b, ct, col, width):
            return st[b]["xf"][:, ct, col:col + width]

        def emit_tr(b, k):
            tp = pT.tile([128, C], BF16, tag="pt", name=f"tp_{b}_{k}")
            for ct in range(CT):
                nc.tensor.transpose(
                    tp[:, ct * 128:(ct + 1) * 128],
                    xf_slice(b, ct, k * 128, 128),
                    ident[:],
                )
            xT = xfT_pool.tile([128, C], BF16, tag="xT", name=f"xT_{b}_{k}")
            if k % 3 == 2:
                nc.vector.tensor_copy(xT[:], tp[:])
            else:
                nc.scalar.copy(xT[:], tp[:])
            return xT

        def emit_mm1(b, k, xT):
            # energy is symmetric: compute only j >= i blocks (shrinking
            # moving width per i-tile); lower blocks are mirrored after
            for it in range(CT):
                nc.tensor.matmul(
                    st[b]["e"][it][:, it * 128:C],
                    xT[:, it * 128:(it + 1) * 128],
                    xT[:, it * 128:C],
                    start=(k == 0),
                    stop=(k == KT - 1),
                )

        def emit_trmm1(b, k_from=0, prefix=()):
            s = st[b]
            s["e"] = [
                pE.tile([128, C], F32, tag="pe", name=f"pe_{b}_{i}")
                for i in range(CT)
            ]
            pending = list(prefix)
            for k in range(k_from, KT):
                pending.append(emit_tr(b, k))
                if len(pending) > 1:
                    emit_mm1(b, k - len(pending) + 1, pending.pop(0))
            base = KT - len(pending)
            for i, xT in enumerate(pending):
                emit_mm1(b, base + i, xT)

        def emit_mirror(b):
            # mirror lower-triangle blocks e[t][:, u] = e[u][:, t].T via
            # sbuf bounce + transpose into a scratch psum bank + DVE
            # write-back (PE never touches accumulation-grouped banks)
            e_ps = st[b]["e"]
            for t in range(1, CT):
                mp = pT.tile([128, C], BF16, tag="pt", name=f"mp_{b}_{t}")
                for u in range(t):
                    mtmp = s_pool.tile([128, 128], BF16, tag="mir",
                                       name=f"mir_{b}_{t}_{u}")
                    nc.vector.tensor_copy(
                        mtmp[:], e_ps[u][:, t * 128:(t + 1) * 128])
                    nc.tensor.transpose(
                        mp[:, u * 128:(u + 1) * 128], mtmp[:], ident[:])
                nc.vector.tensor_copy(
                    e_ps[t][:, 0:t * 128], mp[:, 0:t * 128])

        def emit_softmax(b):
            s = st[b]
            s["att"] = []
            for it in range(CT):
                m = stat_pool.tile([128, 1], F32, tag="m",
                                   name=f"m_{b}_{it}")
                nc.vector.tensor_reduce(
                    m[:], s["e"][it][:], axis=mybir.AxisListType.X,
                    op=mybir.AluOpType.min,
                )
                sx = s_pool.tile([128, C], F32, tag="s", name=f"s_{b}_{it}")
                z = stat_pool.tile([128, 1], F32, tag="z",
                                   name=f"z_{b}_{it}")
                nc.scalar.activation(
                    sx[:], s["e"][it][:], mybir.ActivationFunctionType.Exp,
                    bias=m[:], scale=-1.0, accum_out=z[:],
                )
                rz = stat_pool.tile([128, 1], F32, tag="rz",
                                    name=f"rz_{b}_{it}")
                nc.vector.reciprocal(rz[:], z[:])
                g = stat_pool.tile([128, 1], F32, tag="g",
                                   name=f"g_{b}_{it}")
                nc.vector.tensor_mul(g[:], rz[:], g_bc[:])
                a = att_pool.tile([128, C], BF16, tag="a",
                                  name=f"a_{b}_{it}")
                nc.vector.tensor_scalar_mul(a[:], sx[:], g[:])
                s["att"].append(a)

        def emit_attT(b):
            s = st[b]
            s["attT"] = []
            for jt in range(CT):
                tp = pT.tile([128, C], BF16, tag="pt", name=f"at_{b}_{jt}")
                for it in range(CT):
                    nc.tensor.transpose(
                        tp[:, it * 128:(it + 1) * 128],
                        s["att"][it][:, jt * 128:(jt + 1) * 128],
                        ident[:],
                    )
                aT = attT_pool.tile([128, C], BF16, tag="aT",
                                    name=f"aT_{b}_{jt}")
                nc.scalar.copy(aT[:], tp[:])
                s["attT"].append(aT)

        def emit_mm2(b, its=range(CT), wide_psum=False):
            s = st[b]
            for it in its:
                for h in range(N // 1024):
                    o = out_pool.tile([128, 1024], F32, tag="o",
                                      name=f"o_{b}_{it}_{h}")
                    for sub in range(2):
                        nch = 2 * h + sub
                        if wide_psum and (it * NCH + nch) % 2 == 1:
                            po = pT.tile([128, 512], F32, tag="pt",
                                         name=f"po_{b}_{it}_{nch}")
                        else:
                            po = pO.tile([128, 512], F32, tag="po",
                                         name=f"po_{b}_{it}_{nch}")
                        for jt in range(CT):
                            nc.tensor.matmul(
                                po[:],
                                s["attT"][jt][:, it * 128:(it + 1) * 128],
                                xf_slice(b, jt, nch * 512, 512),
                                start=(jt == 0),
                                stop=(jt == CT - 1),
                            )
                        nc.vector.tensor_add(
                            o[:, sub * 512:(sub + 1) * 512], po[:],
                            xf_slice(b, it, nch * 512, 512),
                        )
                    nc.sync.dma_start(
                        o_d[b, it * 128:(it + 1) * 128,
                            h * 1024:(h + 1) * 1024],
                        o[:],
                    )

        # interleaved emission: batch 1's transposes fill the PE bubble
        # created by batch 0's softmax chain
        PFX = 6
        emit_loads(0)
        emit_trmm1(0)
        emit_mirror(0)
        emit_loads(1)
        prefix = [emit_tr(1, k) for k in range(PFX)]
        emit_softmax(0)
        emit_attT(0)
        emit_mm2(0, its=range(0, CT - 1))
        emit_trmm1(1, k_from=PFX, prefix=prefix)
        emit_mirror(1)
        emit_mm2(0, its=range(CT - 1, CT))
        emit_softmax(1)
        # HAM warmth: batch 1's softmax leaves PE idle for ~3us, right at
        # the clock-gate MID window; a WAW-chained run of tiny matmuls
        # keeps the PE activity monitor busy so the tail MM2 starts at
        # 2.4 GHz instead of re-throttled 1.2 GHz
        warm = pT.tile([128, 64], F32, tag="pt", name="warm")
        for i in range(8):
            nc.tensor.matmul(warm[:], ident[:], ident[:, 0:64],
                             start=True, stop=True)
        emit_attT(1)
        emit_mm2(1, wide_psum=True)

    nc.compile()
    return nc


_RUNNER = None


def _build_runner(nc=None):
    """Compile once; return a callable (xf_full, gamma) -> out_full.

    Mirrors concourse.bass2jax.run_bass_via_pjrt but caches the jitted
    shard_map executable so repeated kernel() calls don't re-lower, and
    keeps the output-seed zero buffers resident on device.
    """
    import jax
    from jax.sharding import Mesh, NamedSharding, PartitionSpec
    from jax.experimental.shard_map import shard_map

    from concourse import bass2jax, mybir as _mybir
    from concourse.bass2jax import _bass_exec_p, partition_id_tensor

    if nc is None:
        nc = _build_nc()
    bass2jax.install_neuronx_cc_hook()

    partition_name = (
        nc.partition_id_tensor.name if nc.partition_id_tensor else None
    )
    in_names, out_names, out_avals, zero_shapes = [], [], [], []
    for alloc in nc.m.functions[0].allocations:
        if not isinstance(alloc, _mybir.MemoryLocationSet):
            continue
        name = alloc.memorylocations[0].name
        if alloc.kind == "ExternalInput":
            if name != partition_name:
                in_names.append(name)
        elif alloc.kind == "ExternalOutput":
            shape = tuple(alloc.tensor_shape)
            dtype = _mybir.dt.np(alloc.dtype)
            out_names.append(name)
            out_avals.append(jax.core.ShapedArray(shape, dtype))
            zero_shapes.append((shape, dtype))
    n_params = len(in_names)
    all_names = list(in_names) + list(out_names)
    if partition_name is not None:
        all_names.append(partition_name)
    donate = tuple(range(n_params, n_params + len(out_names)))

    def _body(*args):
        operands = list(args)
        if partition_name is not None:
            operands.append(partition_id_tensor())
        return tuple(
            _bass_exec_p.bind(
                *operands,
                out_avals=tuple(out_avals),
                in_names=tuple(all_names),
                out_names=tuple(out_names),
                lowering_input_output_aliases=(),
                sim_require_finite=True,
                sim_require_nnan=True,
                nc=nc,
            )
        )

    devices = jax.devices()[:N_CORES]
    mesh = Mesh(np.asarray(devices), ("core",))
    n_in = n_params + len(out_names)
    sharded = jax.jit(
        shard_map(
            _body,
            mesh=mesh,
            in_specs=(PartitionSpec("core"),) * n_in,
            out_specs=(PartitionSpec("core"),) * len(out_names),
            check_rep=False,
        ),
        keep_unused=True,
    )

    # in_names order is discovered from allocations; map our two inputs
    assert set(in_names) == {"x", "gamma"}, in_names

    # output-seed buffers created on device once (kernel writes out fully)
    sh = NamedSharding(mesh, PartitionSpec("core"))
    zeros_dev = [
        jax.jit(
            lambda s=s, d=d: jax.numpy.zeros((N_CORES * s[0],) + s[1:], d),
            out_shardings=sh,
        )()
        for s, d in zero_shapes
    ]
    jax.block_until_ready(zeros_dev)

    def run(xf_full, gamma):
        per_in = {
            "x": xf_full,  # (16, 512, 4096) == concat of per-core (2, 512, 4096)
            "gamma": np.ascontiguousarray(
                np.broadcast_to(np.asarray(gamma, np.float32).reshape(1),
                                (N_CORES,))
            ),
        }
        concat_in = [per_in[name] for name in in_names]
        out_arrs = sharded(*concat_in, *zeros_dev)
        return np.asarray(out_arrs[out_names.index("out")])

    run.sharded = sharded
    run.zeros_dev = zeros_dev
    run.in_names = in_names
    run.out_names = out_names
    run.mesh = mesh
    return run


def _get_runner():
    global _RUNNER
    if _RUNNER is None:
        _RUNNER = _build_runner()
    return _RUNNER


def kernel(x, gamma):
    assert x.shape == (B, C, H, W)
    run = _get_runner()
    xf = np.ascontiguousarray(np.asarray(x, np.float32).reshape(B, C, N))
    g = np.asarray(gamma, np.float32)
    out = run(xf, g)
    return out.reshape(B, C, H, W).astype(np.float32, copy=False)



# revision 56
# speedup vs baseline: 1.1304x; 1.1304x over previous
"""CAM_Module (channel attention) Trainium2 Bass kernel (all-bf16 variant).

x is SWDGE cast-loaded straight to bf16 (the only resident copy): every PE
path (transpose 1 cyc/row, MM1, MM2) runs full-rate bf16, and the residual
carries a one-time bf16 rounding (~2.3e-3 rel vs the 2e-2 gate).

x: (16, 512, 64, 64) f32, gamma: (1,) f32
  xf = x.reshape(B, C, N)           N = 4096
  energy = xf @ xf^T                (B, C, C)
  att = softmax(max(energy) - energy, axis=-1)   == softmax(-energy) (shift-invariant)
  out = gamma * (att @ xf) + x

Sharding: data-parallel over batch, 2 batches per core on 8 cores.

Per-core pipeline (per batch):
  - SWDGE cast-load x -> SBUF bf16 (one 3D DMA per column-chunk covering
    all 4 c-tiles, so compute starts before the full batch arrives)
  - PE transpose-mode (bf16, 1 cyc/row): xf^T chunks -> PSUM -> evac (ACT
    2/3 / DVE 1/3) -> SBUF bf16
  - MM1 (bf16): energy j>=i blocks accumulated over 32 k-chunks into 4
    PSUM banks; lower-triangle blocks mirrored via PE transpose
  - softmax: DVE row-min, ACT exp(min - e) with row-sum accumulation,
    DVE reciprocal, scale by gamma/Z
  - PE transpose att -> att^T (bf16)
  - MM2 (bf16): att^T.T @ xf accumulated over 4 j-chunks, DVE adds the
    (bf16-rounded) residual x
  - HWDGE store f32

Measured on HW (257-rep loop slope, 8 cores): 150.4 us, vs 187.4 us for
the session-start baseline measured the same way (HW has ~5% run-to-run
variance; the previous per-(c-tile,chunk)-load revision measured
153.2/160.9). The two changes that got here from that revision: merged
3D SWDGE cast-loads (5 DMAs/batch instead of 20 - Q7 descgen is per-DMA
dominated, so the head streams at HBM pace), and rotating the tail MM2's
po tiles across the idle pT banks (4-deep PSUM pipeline hides the
DVE-add/semaphore round trip that gates PE at 2 banks). Rejected variants, all
HW-measured: fp8 DoubleRow matmuls (~1.7-1.9x slower than the cost model
predicts: un-hidden 256-col LDWEIGHTS per DR matmul, and fp8
transpose-mode's mandatory step-2 strided PSUM writes -> 185-202 us);
fine-grained or coarse-block emission interleave of one batch's MM2 with
the other's transposes (161-174 us); merged 3D HWDGE loads (167 us);
folding gamma into the MM2 evac + deeper prefix (160 us); paired
transpose evacs (164 us); a PE warmth-filler matmul chain bridging the
batch-1 softmax window (163 us). The remaining gap to the ~94 us DMA floor is
the serial head (batch-0 load) and tail (batch-1 MM2+stores).
"""

import sys

if "/opt/trn_rl_repo" not in sys.path:
    sys.path.insert(0, "/opt/trn_rl_repo")

from contextlib import ExitStack

import numpy as np

import concourse.bass as bass
import concourse.tile as tile
from concourse import bacc, mybir
from concourse.bass_utils import run_bass_kernel_spmd
from concourse.masks import make_identity

N_CORES = 8
B, C, H, W = 16, 512, 64, 64
N = H * W                    # 4096
BPC = B // N_CORES           # batches per core = 2
CT = C // 128                # 4 c-tiles
KT = N // 128                # 32 k-chunks (transposed layout)
NCH = N // 512               # 8 moving chunks for MM2

F32 = mybir.dt.float32
F32R = mybir.dt.float32r
BF16 = mybir.dt.bfloat16


def _build_nc(reps=1, upto="full"):
    nc = bacc.Bacc("TRN2", target_bir_lowering=False, debug=False,
                   num_devices=N_CORES)
    x_d = nc.dram_tensor("x", [BPC, C, N], F32, kind="ExternalInput").ap()
    g_d = nc.dram_tensor("gamma", [1], F32, kind="ExternalInput").ap()
    o_d = nc.dram_tensor("out", [BPC, C, N], F32, kind="ExternalOutput").ap()

    with tile.TileContext(nc) as tc, ExitStack() as ctx:
        xf_pool = ctx.enter_context(tc.tile_pool(name="xf", bufs=BPC))
        xfT_pool = ctx.enter_context(tc.tile_pool(name="xfT", bufs=12))
        s_pool = ctx.enter_context(tc.tile_pool(name="s", bufs=CT))
        att_pool = ctx.enter_context(tc.tile_pool(name="att", bufs=CT))
        attT_pool = ctx.enter_context(tc.tile_pool(name="attT", bufs=2 * CT))
        out_pool = ctx.enter_context(tc.tile_pool(name="outp", bufs=6))
        stat_pool = ctx.enter_context(tc.tile_pool(name="stat", bufs=4 * CT))
        one_pool = ctx.enter_context(tc.tile_pool(name="one", bufs=1))
        pT = ctx.enter_context(tc.tile_pool(name="pT", bufs=2, space="PSUM"))
        pE = ctx.enter_context(tc.tile_pool(name="pE", bufs=CT, space="PSUM"))
        pO = ctx.enter_context(tc.tile_pool(name="pO", bufs=2, space="PSUM"))

        # identity for PE transpose-mode (f32r so dtypes match the data)
        ident_f = one_pool.tile([128, 128], F32, tag="idf")
        make_identity(nc, ident_f[:])
        ident = one_pool.tile([128, 128], BF16, tag="idb")
        nc.vector.tensor_copy(ident[:], ident_f[:])

        # broadcast gamma to all 128 partitions via K=1 matmul with ones
        g_sb = one_pool.tile([1, 1], F32, tag="gsb")
        nc.sync.dma_start(g_sb[:], g_d.rearrange("(a b) -> a b", a=1))
        ones = one_pool.tile([1, 128], F32, tag="ones")
        nc.vector.memset(ones[:], 1.0)
        pG = pT.tile([128, 1], F32, tag="pt", name="pG")
        nc.tensor.matmul(pG[:], ones[:], g_sb[:], start=True, stop=True)
        g_bc = one_pool.tile([128, 1], F32, tag="gbc")
        nc.vector.tensor_copy(g_bc[:], pG[:])

        loop_ctx = tc.For_i(0, reps, 1) if reps > 1 else None
        if loop_ctx is not None:
            ctx.enter_context(loop_ctx)

        # per-c-tile load chunks: a small first chunk so the pipeline
        # starts early, bigger ones later (amortize SWDGE fixed cost)
        CHUNKS = [(0, 512), (512, 512), (1024, 1024), (2048, 1024),
                  (3072, 1024)]

        def chunk_of(col):
            for i, (off, w) in enumerate(CHUNKS):
                if off <= col < off + w:
                    return i, col - off
            raise AssertionError(col)

        st = [dict() for _ in range(BPC)]

        def emit_loads(b):
            # one bf16 tile per batch; each chunk is a single 3D SWDGE
            # cast-DMA covering all 4 c-tiles: Q7 descgen cost is per-DMA
            # dominated, so 5 DMAs stream at HBM pace where 20 were
            # descgen-paced
            s = st[b]
            t = xf_pool.tile([128, CT, N], BF16, tag="xf", name=f"xf_{b}")
            src = x_d[b].rearrange("(ct p) n -> p ct n", p=128)
            for q in range(len(CHUNKS)):
                off, w = CHUNKS[q]
                nc.gpsimd.dma_start(t[:, :, off:off + w],
                                    src[:, :, off:off + w])
            s["xf"] = t

        def xf_slice(b, ct, col, width):
            return st[b]["xf"][:, ct, col:col + width]

        def emit_tr(b, k):
            tp = pT.tile([128, C], BF16, tag="pt", name=f"tp_{b}_{k}")
            for ct in range(CT):
                nc.tensor.transpose(
                    tp[:, ct * 128:(ct + 1) * 128],
                    xf_slice(b, ct, k * 128, 128),
                    ident[:],
                )
            xT = xfT_pool.tile([128, C], BF16, tag="xT", name=f"xT_{b}_{k}")
            if k % 3 == 2:
                nc.vector.tensor_copy(xT[:], tp[:])
            else:
                nc.scalar.copy(xT[:], tp[:])
            return xT

        def emit_mm1(b, k, xT):
            # energy is symmetric: compute only j >= i blocks (shrinking
            # moving width per i-tile); lower blocks are mirrored after
            for it in range(CT):
                nc.tensor.matmul(
                    st[b]["e"][it][:, it * 128:C],
                    xT[:, it * 128:(it + 1) * 128],
                    xT[:, it * 128:C],
                    start=(k == 0),
                    stop=(k == KT - 1),
                )

        def emit_trmm1(b, k_from=0, prefix=()):
            s = st[b]
            s["e"] = [
                pE.tile([128, C], F32, tag="pe", name=f"pe_{b}_{i}")
                for i in range(CT)
            ]
            pending = list(prefix)
            for k in range(k_from, KT):
                pending.append(emit_tr(b, k))
                if len(pending) > 1:
                    emit_mm1(b, k - len(pending) + 1, pending.pop(0))
            base = KT - len(pending)
            for i, xT in enumerate(pending):
                emit_mm1(b, base + i, xT)

        def emit_mirror(b):
            # mirror lower-triangle blocks e[t][:, u] = e[u][:, t].T via
            # sbuf bounce + transpose into a scratch psum bank + DVE
            # write-back (PE never touches accumulation-grouped banks)
            e_ps = st[b]["e"]
            for t in range(1, CT):
                mp = pT.tile([128, C], BF16, tag="pt", name=f"mp_{b}_{t}")
                for u in range(t):
                    mtmp = s_pool.tile([128, 128], BF16, tag="mir",
                                       name=f"mir_{b}_{t}_{u}")
                    nc.vector.tensor_copy(
                        mtmp[:], e_ps[u][:, t * 128:(t + 1) * 128])
                    nc.tensor.transpose(
                        mp[:, u * 128:(u + 1) * 128], mtmp[:], ident[:])
                nc.vector.tensor_copy(
                    e_ps[t][:, 0:t * 128], mp[:, 0:t * 128])

        def emit_softmax(b):
            s = st[b]
            s["att"] = []
            for it in range(CT):
                m = stat_pool.tile([128, 1], F32, tag="m",
                                   name=f"m_{b}_{it}")
                nc.vector.tensor_reduce(
                    m[:], s["e"][it][:], axis=mybir.AxisListType.X,
                    op=mybir.AluOpType.min,
                )
                sx = s_pool.tile([128, C], F32, tag="s", name=f"s_{b}_{it}")
                z = stat_pool.tile([128, 1], F32, tag="z",
                                   name=f"z_{b}_{it}")
                nc.scalar.activation(
                    sx[:], s["e"][it][:], mybir.ActivationFunctionType.Exp,
                    bias=m[:], scale=-1.0, accum_out=z[:],
                )
                rz = stat_pool.tile([128, 1], F32, tag="rz",
                                    name=f"rz_{b}_{it}")
                nc.vector.reciprocal(rz[:], z[:])
                g = stat_pool.tile([128, 1], F32, tag="g",
                                   name=f"g_{b}_{it}")
                nc.vector.tensor_mul(g[:], rz[:], g_bc[:])
                a = att_pool.tile([128, C], BF16, tag="a",
                                  name=f"a_{b}_{it}")
                nc.vector.tensor_scalar_mul(a[:], sx[:], g[:])
                s["att"].append(a)

        def emit_attT(b):
            s = st[b]
            s["attT"] = []
            for jt in range(CT):
                tp = pT.tile([128, C], BF16, tag="pt", name=f"at_{b}_{jt}")
                for it in range(CT):
                    nc.tensor.transpose(
                        tp[:, it * 128:(it + 1) * 128],
                        s["att"][it][:, jt * 128:(jt + 1) * 128],
                        ident[:],
                    )
                aT = attT_pool.tile([128, C], BF16, tag="aT",
                                    name=f"aT_{b}_{jt}")
                nc.scalar.copy(aT[:], tp[:])
                s["attT"].append(aT)

        def emit_mm2(b, its=range(CT), wide_psum=False):
            s = st[b]
            for it in its:
                for h in range(N // 1024):
                    o = out_pool.tile([128, 1024], F32, tag="o",
                                      name=f"o_{b}_{it}_{h}")
                    for sub in range(2):
                        nch = 2 * h + sub
                        if wide_psum and (it * NCH + nch) % 2 == 1:
                            po = pT.tile([128, 512], F32, tag="pt",
                                         name=f"po_{b}_{it}_{nch}")
                        else:
                            po = pO.tile([128, 512], F32, tag="po",
                                         name=f"po_{b}_{it}_{nch}")
                        for jt in range(CT):
                            nc.tensor.matmul(
                                po[:],
                                s["attT"][jt][:, it * 128:(it + 1) * 128],
                                xf_slice(b, jt, nch * 512, 512),
                                start=(jt == 0),
                                stop=(jt == CT - 1),
                            )
                        nc.vector.tensor_add(
                            o[:, sub * 512:(sub + 1) * 512], po[:],
                            xf_slice(b, it, nch * 512, 512),
                        )
                    nc.sync.dma_start(
                        o_d[b, it * 128:(it + 1) * 128,
                            h * 1024:(h + 1) * 1024],
                        o[:],
                    )

        # interleaved emission: batch 1's transposes fill the PE bubble
        # created by batch 0's softmax chain
        # ordering rationale: batch 1's MM1 runs immediately after batch
        # 0's softmax (its loads complete around then), so the serial
        # MM1->softmax->MM2 chain of the LAST batch ends as early as
        # possible; both MM2 phases then run back-to-back on PE with the
        # tail's 4-bank PSUM rotation. softmax1's DVE ops are emitted
        # before the 64 MM2 residual adds so the DVE FIFO cannot
        # serialize them behind ~40us of adds.
        PFX = 6
        emit_loads(0)
        emit_trmm1(0)
        emit_mirror(0)
        emit_loads(1)
        prefix = [emit_tr(1, k) for k in range(PFX)]
        emit_softmax(0)
        emit_attT(0)
        emit_trmm1(1, k_from=PFX, prefix=prefix)
        emit_mirror(1)
        emit_softmax(1)
        emit_attT(1)
        emit_mm2(0, wide_psum=True)
        emit_mm2(1, wide_psum=True)

    nc.compile()
    return nc


_RUNNER = None


def _build_runner(nc=None):
    """Compile once; return a callable (xf_full, gamma) -> out_full.

    Mirrors concourse.bass2jax.run_bass_via_pjrt but caches the jitted
    shard_map executable so repeated kernel() calls don't re-lower, and
    keeps the output-seed zero buffers resident on device.
    """
    import jax
    from jax.sharding import Mesh, NamedSharding, PartitionSpec
    from jax.experimental.shard_map import shard_map

    from concourse import bass2jax, mybir as _mybir
    from concourse.bass2jax import _bass_exec_p, partition_id_tensor

    if nc is None:
        nc = _build_nc()
    bass2jax.install_neuronx_cc_hook()

    partition_name = (
        nc.partition_id_tensor.name if nc.partition_id_tensor else None
    )
    in_names, out_names, out_avals, zero_shapes = [], [], [], []
    for alloc in nc.m.functions[0].allocations:
        if not isinstance(alloc, _mybir.MemoryLocationSet):
            continue
        name = alloc.memorylocations[0].name
        if alloc.kind == "ExternalInput":
            if name != partition_name:
                in_names.append(name)
        elif alloc.kind == "ExternalOutput":
            shape = tuple(alloc.tensor_shape)
            dtype = _mybir.dt.np(alloc.dtype)
            out_names.append(name)
            out_avals.append(jax.core.ShapedArray(shape, dtype))
            zero_shapes.append((shape, dtype))
    n_params = len(in_names)
    all_names = list(in_names) + list(out_names)
    if partition_name is not None:
        all_names.append(partition_name)
    donate = tuple(range(n_params, n_params + len(out_names)))

    def _body(*args):
        operands = list(args)
        if partition_name is not None:
            operands.append(partition_id_tensor())
        return tuple(
            _bass_exec_p.bind(
                *operands,
                out_avals=tuple(out_avals),
                in_names=tuple(all_names),
                out_names=tuple(out_names),
                lowering_input_output_aliases=(),
                sim_require_finite=True,
                sim_require_nnan=True,
                nc=nc,
            )
        )

    devices = jax.devices()[:N_CORES]
    mesh = Mesh(np.asarray(devices), ("core",))
    n_in = n_params + len(out_names)
    sharded = jax.jit(
        shard_map(
            _body,
            mesh=mesh,
            in_specs=(PartitionSpec("core"),) * n_in,
            out_specs=(PartitionSpec("core"),) * len(out_names),
            check_rep=False,
        ),
        keep_unused=True,
    )

    # in_names order is discovered from allocations; map our two inputs
    assert set(in_names) == {"x", "gamma"}, in_names

    # output-seed buffers created on device once (kernel writes out fully)
    sh = NamedSharding(mesh, PartitionSpec("core"))
    zeros_dev = [
        jax.jit(
            lambda s=s, d=d: jax.numpy.zeros((N_CORES * s[0],) + s[1:], d),
            out_shardings=sh,
        )()
        for s, d in zero_shapes
    ]
    jax.block_until_ready(zeros_dev)

    def run(xf_full, gamma):
        per_in = {
            "x": xf_full,  # (16, 512, 4096) == concat of per-core (2, 512, 4096)
            "gamma": np.ascontiguousarray(
                np.broadcast_to(np.asarray(gamma, np.float32).reshape(1),
                                (N_CORES,))
            ),
        }
        concat_in = [per_in[name] for name in in_names]
        out_arrs = sharded(*concat_in, *zeros_dev)
        return np.asarray(out_arrs[out_names.index("out")])

    run.sharded = sharded
    run.zeros_dev = zeros_dev
    run.in_names = in_names
    run.out_names = out_names
    run.mesh = mesh
    return run


def _get_runner():
    global _RUNNER
    if _RUNNER is None:
        _RUNNER = _build_runner()
    return _RUNNER


def kernel(x, gamma):
    assert x.shape == (B, C, H, W)
    run = _get_runner()
    xf = np.ascontiguousarray(np.asarray(x, np.float32).reshape(B, C, N))
    g = np.asarray(gamma, np.float32)
    out = run(xf, g)
    return out.reshape(B, C, H, W).astype(np.float32, copy=False)



# revision 59
# speedup vs baseline: 1.1623x; 1.0283x over previous
"""CAM_Module (channel attention) Trainium2 Bass kernel (all-bf16 variant).

x is SWDGE cast-loaded straight to bf16 (the only resident copy): every PE
path (transpose 1 cyc/row, MM1, MM2) runs full-rate bf16, and the residual
carries a one-time bf16 rounding (~2.3e-3 rel vs the 2e-2 gate).

x: (16, 512, 64, 64) f32, gamma: (1,) f32
  xf = x.reshape(B, C, N)           N = 4096
  energy = xf @ xf^T                (B, C, C)
  att = softmax(max(energy) - energy, axis=-1)   == softmax(-energy) (shift-invariant)
  out = gamma * (att @ xf) + x

Sharding: data-parallel over batch, 2 batches per core on 8 cores.

Per-core pipeline (per batch):
  - SWDGE cast-load x -> SBUF bf16 (one 3D DMA per column-chunk covering
    all 4 c-tiles, so compute starts before the full batch arrives)
  - PE transpose-mode (bf16, 1 cyc/row): xf^T chunks -> PSUM -> evac (ACT
    2/3 / DVE 1/3) -> SBUF bf16
  - MM1 (bf16): energy j>=i blocks accumulated over 32 k-chunks into 4
    PSUM banks; lower-triangle blocks mirrored via PE transpose
  - softmax: DVE row-min, ACT exp(min - e) with row-sum accumulation,
    DVE reciprocal, scale by gamma/Z
  - PE transpose att -> att^T (bf16)
  - MM2 (bf16): att^T.T @ xf accumulated over 4 j-chunks, DVE adds the
    (bf16-rounded) residual x
  - HWDGE store f32

Measured on HW (257-rep loop slope, 8 cores): 144.1 us, vs 187.4 us for
the session-start baseline measured the same way (HW has ~5% run-to-run
variance; earlier revisions of this dtype scheme measured 150.4 and
153.2/160.9). The changes that got here: (1) merged 3D SWDGE cast-loads
(5 DMAs/batch instead of 20 - Q7 descgen is per-DMA dominated, so the
head streams at HBM pace); (2) batch-1's MM1 runs IMMEDIATELY after
batch-0's softmax instead of behind batch-0's MM2 in the PE FIFO, so the
last batch's serial MM1->softmax->MM2 chain ends as early as possible,
with softmax1's DVE ops emitted before the 64 MM2 residual adds (else
the DVE FIFO serializes them behind ~40us of adds) and attT double-
buffered across batches (else a PSUM-slot WAR cycle deadlocks); (3) both
MM2 phases then run back-to-back with po tiles rotating across the idle
pT banks (4-deep PSUM pipeline hides the DVE-add/sem round trip that
gates PE at 2 banks). Rejected variants, all
HW-measured: fp8 DoubleRow matmuls (~1.7-1.9x slower than the cost model
predicts: un-hidden 256-col LDWEIGHTS per DR matmul, and fp8
transpose-mode's mandatory step-2 strided PSUM writes -> 185-202 us);
fine-grained or coarse-block emission interleave of one batch's MM2 with
the other's transposes (161-174 us); merged 3D HWDGE loads (167 us);
folding gamma into the MM2 evac + deeper prefix (160 us); paired
transpose evacs (164 us); a PE warmth-filler matmul chain bridging the
batch-1 softmax window (163 us). The remaining gap to the ~94 us DMA floor is
the serial head (batch-0 load) and tail (batch-1 MM2+stores).
"""

import sys

if "/opt/trn_rl_repo" not in sys.path:
    sys.path.insert(0, "/opt/trn_rl_repo")

from contextlib import ExitStack

import numpy as np

import concourse.bass as bass
import concourse.tile as tile
from concourse import bacc, mybir
from concourse.bass_utils import run_bass_kernel_spmd
from concourse.masks import make_identity

N_CORES = 8
B, C, H, W = 16, 512, 64, 64
N = H * W                    # 4096
BPC = B // N_CORES           # batches per core = 2
CT = C // 128                # 4 c-tiles
KT = N // 128                # 32 k-chunks (transposed layout)
NCH = N // 512               # 8 moving chunks for MM2

F32 = mybir.dt.float32
F32R = mybir.dt.float32r
BF16 = mybir.dt.bfloat16


def _build_nc(reps=1, upto="full"):
    nc = bacc.Bacc("TRN2", target_bir_lowering=False, debug=False,
                   num_devices=N_CORES)
    x_d = nc.dram_tensor("x", [BPC, C, N], F32, kind="ExternalInput").ap()
    g_d = nc.dram_tensor("gamma", [1], F32, kind="ExternalInput").ap()
    o_d = nc.dram_tensor("out", [BPC, C, N], F32, kind="ExternalOutput").ap()

    with tile.TileContext(nc) as tc, ExitStack() as ctx:
        xf_pool = ctx.enter_context(tc.tile_pool(name="xf", bufs=BPC))
        xfT_pool = ctx.enter_context(tc.tile_pool(name="xfT", bufs=12))
        s_pool = ctx.enter_context(tc.tile_pool(name="s", bufs=CT))
        att_pool = ctx.enter_context(tc.tile_pool(name="att", bufs=CT))
        attT_pool = ctx.enter_context(tc.tile_pool(name="attT", bufs=2 * CT))
        out_pool = ctx.enter_context(tc.tile_pool(name="outp", bufs=6))
        stat_pool = ctx.enter_context(tc.tile_pool(name="stat", bufs=4 * CT))
        one_pool = ctx.enter_context(tc.tile_pool(name="one", bufs=1))
        pT = ctx.enter_context(tc.tile_pool(name="pT", bufs=2, space="PSUM"))
        pE = ctx.enter_context(tc.tile_pool(name="pE", bufs=CT, space="PSUM"))
        pO = ctx.enter_context(tc.tile_pool(name="pO", bufs=2, space="PSUM"))

        # identity for PE transpose-mode (f32r so dtypes match the data)
        ident_f = one_pool.tile([128, 128], F32, tag="idf")
        make_identity(nc, ident_f[:])
        ident = one_pool.tile([128, 128], BF16, tag="idb")
        nc.vector.tensor_copy(ident[:], ident_f[:])

        # broadcast gamma to all 128 partitions via K=1 matmul with ones
        g_sb = one_pool.tile([1, 1], F32, tag="gsb")
        nc.sync.dma_start(g_sb[:], g_d.rearrange("(a b) -> a b", a=1))
        ones = one_pool.tile([1, 128], F32, tag="ones")
        nc.vector.memset(ones[:], 1.0)
        pG = pT.tile([128, 1], F32, tag="pt", name="pG")
        nc.tensor.matmul(pG[:], ones[:], g_sb[:], start=True, stop=True)
        g_bc = one_pool.tile([128, 1], F32, tag="gbc")
        nc.vector.tensor_copy(g_bc[:], pG[:])

        loop_ctx = tc.For_i(0, reps, 1) if reps > 1 else None
        if loop_ctx is not None:
            ctx.enter_context(loop_ctx)

        # per-c-tile load chunks: a small first chunk so the pipeline
        # starts early, bigger ones later (amortize SWDGE fixed cost)
        CHUNKS = [(0, 512), (512, 512), (1024, 1024), (2048, 1024),
                  (3072, 1024)]

        def chunk_of(col):
            for i, (off, w) in enumerate(CHUNKS):
                if off <= col < off + w:
                    return i, col - off
            raise AssertionError(col)

        st = [dict() for _ in range(BPC)]

        def emit_loads(b):
            # one bf16 tile per batch; each chunk is a single 3D SWDGE
            # cast-DMA covering all 4 c-tiles: Q7 descgen cost is per-DMA
            # dominated, so 5 DMAs stream at HBM pace where 20 were
            # descgen-paced
            s = st[b]
            t = xf_pool.tile([128, CT, N], BF16, tag="xf", name=f"xf_{b}")
            src = x_d[b].rearrange("(ct p) n -> p ct n", p=128)
            for q in range(len(CHUNKS)):
                off, w = CHUNKS[q]
                nc.gpsimd.dma_start(t[:, :, off:off + w],
                                    src[:, :, off:off + w])
            s["xf"] = t

        def xf_slice(b, ct, col, width):
            return st[b]["xf"][:, ct, col:col + width]

        def emit_tr(b, k):
            # rotate transpose scratch across pT AND pO: in this schedule
            # no MM2 runs concurrently with the transpose streams, so the
            # pO banks are idle here, and 4 banks make the stream PE-paced
            # instead of evac-round-trip-paced (2 banks = ~1us/k-chunk)
            pool, tag = (pT, "pt") if k % 2 == 0 else (pO, "po")
            tp = pool.tile([128, C], BF16, tag=tag, name=f"tp_{b}_{k}")
            for ct in range(CT):
                nc.tensor.transpose(
                    tp[:, ct * 128:(ct + 1) * 128],
                    xf_slice(b, ct, k * 128, 128),
                    ident[:],
                )
            xT = xfT_pool.tile([128, C], BF16, tag="xT", name=f"xT_{b}_{k}")
            if k % 3 == 2:
                nc.vector.tensor_copy(xT[:], tp[:])
            else:
                nc.scalar.copy(xT[:], tp[:])
            return xT

        def emit_mm1(b, k, xT):
            # energy is symmetric: compute only j >= i blocks (shrinking
            # moving width per i-tile); lower blocks are mirrored after
            for it in range(CT):
                nc.tensor.matmul(
                    st[b]["e"][it][:, it * 128:C],
                    xT[:, it * 128:(it + 1) * 128],
                    xT[:, it * 128:C],
                    start=(k == 0),
                    stop=(k == KT - 1),
                )

        def emit_trmm1(b, k_from=0, prefix=(), k_to=KT, finish=None):
            s = st[b]
            if finish is None:
                s["e"] = [
                    pE.tile([128, C], F32, tag="pe", name=f"pe_{b}_{i}")
                    for i in range(CT)
                ]
            pending = list(prefix)
            for k in range(k_from, k_to):
                pending.append(emit_tr(b, k))
                if len(pending) > 1:
                    emit_mm1(b, k - len(pending) + 1, pending.pop(0))
            if k_to < KT:
                return pending  # caller resumes with finish=True
            base = KT - len(pending)
            for i, xT in enumerate(pending):
                emit_mm1(b, base + i, xT)

        def emit_mirror(b):
            # mirror lower-triangle blocks e[t][:, u] = e[u][:, t].T via
            # sbuf bounce + transpose into a scratch psum bank + DVE
            # write-back (PE never touches accumulation-grouped banks)
            e_ps = st[b]["e"]
            for t in range(1, CT):
                mp = pT.tile([128, C], BF16, tag="pt", name=f"mp_{b}_{t}")
                for u in range(t):
                    mtmp = s_pool.tile([128, 128], BF16, tag="mir",
                                       name=f"mir_{b}_{t}_{u}")
                    nc.vector.tensor_copy(
                        mtmp[:], e_ps[u][:, t * 128:(t + 1) * 128])
                    nc.tensor.transpose(
                        mp[:, u * 128:(u + 1) * 128], mtmp[:], ident[:])
                nc.vector.tensor_copy(
                    e_ps[t][:, 0:t * 128], mp[:, 0:t * 128])

        def emit_softmax(b):
            s = st[b]
            s["att"] = []
            for it in range(CT):
                m = stat_pool.tile([128, 1], F32, tag="m",
                                   name=f"m_{b}_{it}")
                nc.vector.tensor_reduce(
                    m[:], s["e"][it][:], axis=mybir.AxisListType.X,
                    op=mybir.AluOpType.min,
                )
                sx = s_pool.tile([128, C], F32, tag="s", name=f"s_{b}_{it}")
                z = stat_pool.tile([128, 1], F32, tag="z",
                                   name=f"z_{b}_{it}")
                nc.scalar.activation(
                    sx[:], s["e"][it][:], mybir.ActivationFunctionType.Exp,
                    bias=m[:], scale=-1.0, accum_out=z[:],
                )
                rz = stat_pool.tile([128, 1], F32, tag="rz",
                                    name=f"rz_{b}_{it}")
                nc.vector.reciprocal(rz[:], z[:])
                g = stat_pool.tile([128, 1], F32, tag="g",
                                   name=f"g_{b}_{it}")
                nc.vector.tensor_mul(g[:], rz[:], g_bc[:])
                a = att_pool.tile([128, C], BF16, tag="a",
                                  name=f"a_{b}_{it}")
                nc.vector.tensor_scalar_mul(a[:], sx[:], g[:])
                s["att"].append(a)

        def emit_attT(b):
            s = st[b]
            s["attT"] = []
            for jt in range(CT):
                pool, tag = (pT, "pt") if jt % 2 == 0 else (pO, "po")
                tp = pool.tile([128, C], BF16, tag=tag, name=f"at_{b}_{jt}")
                for it in range(CT):
                    nc.tensor.transpose(
                        tp[:, it * 128:(it + 1) * 128],
                        s["att"][it][:, jt * 128:(jt + 1) * 128],
                        ident[:],
                    )
                aT = attT_pool.tile([128, C], BF16, tag="aT",
                                    name=f"aT_{b}_{jt}")
                nc.scalar.copy(aT[:], tp[:])
                s["attT"].append(aT)

        def emit_mm2(b, its=range(CT), wide_psum=False):
            s = st[b]
            for it in its:
                for h in range(N // 1024):
                    o = out_pool.tile([128, 1024], F32, tag="o",
                                      name=f"o_{b}_{it}_{h}")
                    for sub in range(2):
                        nch = 2 * h + sub
                        if wide_psum and (it * NCH + nch) % 2 == 1:
                            po = pT.tile([128, 512], F32, tag="pt",
                                         name=f"po_{b}_{it}_{nch}")
                        else:
                            po = pO.tile([128, 512], F32, tag="po",
                                         name=f"po_{b}_{it}_{nch}")
                        for jt in range(CT):
                            nc.tensor.matmul(
                                po[:],
                                s["attT"][jt][:, it * 128:(it + 1) * 128],
                                xf_slice(b, jt, nch * 512, 512),
                                start=(jt == 0),
                                stop=(jt == CT - 1),
                            )
                        nc.vector.tensor_add(
                            o[:, sub * 512:(sub + 1) * 512], po[:],
                            xf_slice(b, it, nch * 512, 512),
                        )
                    nc.sync.dma_start(
                        o_d[b, it * 128:(it + 1) * 128,
                            h * 1024:(h + 1) * 1024],
                        o[:],
                    )

        # interleaved emission: batch 1's transposes fill the PE bubble
        # created by batch 0's softmax chain
        # ordering rationale: batch 1's MM1 runs immediately after batch
        # 0's softmax (its loads complete around then), so the serial
        # MM1->softmax->MM2 chain of the LAST batch ends as early as
        # possible; both MM2 phases then run back-to-back on PE with the
        # tail's 4-bank PSUM rotation. softmax1's DVE ops are emitted
        # before the 64 MM2 residual adds so the DVE FIFO cannot
        # serialize them behind ~40us of adds.
        PFX = 10
        emit_loads(0)
        emit_trmm1(0)
        emit_mirror(0)
        emit_loads(1)
        prefix = [emit_tr(1, k) for k in range(PFX)]
        emit_softmax(0)
        emit_attT(0)
        emit_trmm1_part1 = emit_trmm1  # alias for clarity
        # k-steps PFX..23 first; while the last load chunk arrives, run
        # MM2_0's first it-block (its inputs have been ready since attT0),
        # then finish the k-stream
        KSPLIT = 24
        carry = emit_trmm1(1, k_from=PFX, prefix=prefix, k_to=KSPLIT)
        emit_mm2(0, its=[0])
        emit_trmm1(1, k_from=KSPLIT, prefix=carry, finish=True)
        emit_mirror(1)
        emit_softmax(1)
        emit_attT(1)
        emit_mm2(0, its=range(1, CT), wide_psum=True)
        emit_mm2(1, wide_psum=True)

    nc.compile()
    return nc


_RUNNER = None


def _build_runner(nc=None):
    """Compile once; return a callable (xf_full, gamma) -> out_full.

    Mirrors concourse.bass2jax.run_bass_via_pjrt but caches the jitted
    shard_map executable so repeated kernel() calls don't re-lower, and
    keeps the output-seed zero buffers resident on device.
    """
    import jax
    from jax.sharding import Mesh, NamedSharding, PartitionSpec
    from jax.experimental.shard_map import shard_map

    from concourse import bass2jax, mybir as _mybir
    from concourse.bass2jax import _bass_exec_p, partition_id_tensor

    if nc is None:
        nc = _build_nc()
    bass2jax.install_neuronx_cc_hook()

    partition_name = (
        nc.partition_id_tensor.name if nc.partition_id_tensor else None
    )
    in_names, out_names, out_avals, zero_shapes = [], [], [], []
    for alloc in nc.m.functions[0].allocations:
        if not isinstance(alloc, _mybir.MemoryLocationSet):
            continue
        name = alloc.memorylocations[0].name
        if alloc.kind == "ExternalInput":
            if name != partition_name:
                in_names.append(name)
        elif alloc.kind == "ExternalOutput":
            shape = tuple(alloc.tensor_shape)
            dtype = _mybir.dt.np(alloc.dtype)
            out_names.append(name)
            out_avals.append(jax.core.ShapedArray(shape, dtype))
            zero_shapes.append((shape, dtype))
    n_params = len(in_names)
    all_names = list(in_names) + list(out_names)
    if partition_name is not None:
        all_names.append(partition_name)
    donate = tuple(range(n_params, n_params + len(out_names)))

    def _body(*args):
        operands = list(args)
        if partition_name is not None:
            operands.append(partition_id_tensor())
        return tuple(
            _bass_exec_p.bind(
                *operands,
                out_avals=tuple(out_avals),
                in_names=tuple(all_names),
                out_names=tuple(out_names),
                lowering_input_output_aliases=(),
                sim_require_finite=True,
                sim_require_nnan=True,
                nc=nc,
            )
        )

    devices = jax.devices()[:N_CORES]
    mesh = Mesh(np.asarray(devices), ("core",))
    n_in = n_params + len(out_names)
    sharded = jax.jit(
        shard_map(
            _body,
            mesh=mesh,
            in_specs=(PartitionSpec("core"),) * n_in,
            out_specs=(PartitionSpec("core"),) * len(out_names),
            check_rep=False,
        ),
        keep_unused=True,
    )

    # in_names order is discovered from allocations; map our two inputs
    assert set(in_names) == {"x", "gamma"}, in_names

    # output-seed buffers created on device once (kernel writes out fully)
    sh = NamedSharding(mesh, PartitionSpec("core"))
    zeros_dev = [
        jax.jit(
            lambda s=s, d=d: jax.numpy.zeros((N_CORES * s[0],) + s[1:], d),
            out_shardings=sh,
        )()
        for s, d in zero_shapes
    ]
    jax.block_until_ready(zeros_dev)

    def run(xf_full, gamma):
        per_in = {
            "x": xf_full,  # (16, 512, 4096) == concat of per-core (2, 512, 4096)
            "gamma": np.ascontiguousarray(
                np.broadcast_to(np.asarray(gamma, np.float32).reshape(1),
                                (N_CORES,))
            ),
        }
        concat_in = [per_in[name] for name in in_names]
        out_arrs = sharded(*concat_in, *zeros_dev)
        return np.asarray(out_arrs[out_names.index("out")])

    run.sharded = sharded
    run.zeros_dev = zeros_dev
    run.in_names = in_names
    run.out_names = out_names
    run.mesh = mesh
    return run


def _get_runner():
    global _RUNNER
    if _RUNNER is None:
        _RUNNER = _build_runner()
    return _RUNNER


def kernel(x, gamma):
    assert x.shape == (B, C, H, W)
    run = _get_runner()
    xf = np.ascontiguousarray(np.asarray(x, np.float32).reshape(B, C, N))
    g = np.asarray(gamma, np.float32)
    out = run(xf, g)
    return out.reshape(B, C, H, W).astype(np.float32, copy=False)

